# revision 11
# baseline (speedup 1.0000x reference)
"""Trainium2 Bass kernel for nn_BinaryConnectNet (binary CNN, 8 NeuronCores).

v2: group-pipelined conv (4 groups of 32 imgs), fp8 DoubleRow depthwise,
max-before-sign pooling, batched DMA, split AllGather (2 halves, g=4
replica groups) overlapped with conv of the second half.

Numerics (all exact vs fp32 reference up to fp32-accumulation rounding):
 - conv1: dense 3x3, K=81 (27 taps x triple-bf16 split of x), one matmul per
   pooling quadrant pair; pool1 = max on raw PSUM (sign is monotonic), then
   one sign+bias eviction per pooled tile.
 - conv2 dw: 6 vertical fp8 DoubleRow tap-pairs (3rd row padded with a
   zero-weight 4th row) accumulated in PSUM; +-1 activations exact in fp8.
 - conv2 pw: K=128 fp8 matmul; dw bias folded into the pw bias; pool2 =
   max-first as above.
 - fc1: fp16 hi/lo weight split (exact to 2^-22), K=16384 over gathered
   fp8 +-1 activations; features sharded 4-way inside each 4-core group.
 - fc2: fp16 hi/lo, per-core partial logits summed on host.
"""

import sys

for _p in ("/opt/trn_rl_repo",):
    if _p not in sys.path:
        sys.path.insert(0, _p)

import numpy as np
import ml_dtypes
from contextlib import ExitStack

import concourse.bass as bass
import concourse.bacc as bacc
import concourse.mybir as mybir
import concourse.tile as tile
from concourse.bass_utils import run_bass_kernel_spmd

F32 = mybir.dt.float32
BF16 = mybir.dt.bfloat16
FP16 = mybir.dt.float16
FP8 = mybir.dt.float8e4
AF = mybir.ActivationFunctionType
ALU = mybir.AluOpType
DR = mybir.MatmulPerfMode.DoubleRow

NCORES = 8
B = 128                 # images per core
GRP = 32                # images per conv pipeline group
NGRP = B // GRP         # 4
H = 32
CHUNK = 2 * 34 * GRP    # 2176 elems per (hc, grp) im2col row
NHC = 16                # pooled rows after pool1
# h1 activation layout: (y 19, x 18, b 32), fp8; y pad rows 0,17,18; x pad 0,17
H1Y, H1X = 19, 18
H1ROW = H1X * GRP       # 576 = x-stride pitch of one y row
H1SZ = H1Y * H1ROW      # 10944
DWSZ = 16 * 16 * GRP    # 8192 dwc elems (y, x, b)
NF1 = 1024
GSIZE = 8               # cores per fc group (full allgather)
NBG = GSIZE * B         # 1024 imgs
FPC = NF1 // GSIZE      # 128 features per core
KFC = 256 * 64          # 16384
NKT = KFC // 128        # 128
NBLK = 16               # fc1 weight DMA blocks (8 k-tiles each)
NB_ALL = NCORES * B


def _bf16(a):
    return np.asarray(a, dtype=ml_dtypes.bfloat16)


def _fp8(a):
    return np.asarray(a, dtype=ml_dtypes.float8_e4m3fn)


def _host_prep(x, w1_dw, b1_dw, w1_pw, b1_pw, w2_dw, b2_dw, w2_pw, b2_pw,
               fc1_w, fc1_b, fc2_w, fc2_b):
    sgn = np.sign
    x = np.asarray(x, np.float32).reshape(NCORES, NGRP, GRP, 3, H, H)

    # triple bf16 split
    x0 = _bf16(x)
    r1 = x - x0.astype(np.float32)
    x1 = _bf16(r1)
    x2 = _bf16(r1 - x1.astype(np.float32))
    splits = [x0, x1, x2]

    # padded per (s, c): [core, 34, 36, grp, b]
    xpad = np.zeros((3, 3, NCORES, H + 2, H + 4, NGRP, GRP),
                    dtype=ml_dtypes.bfloat16)
    for s in range(3):
        for c in range(3):
            xpad[s, c][:, 1:33, 1:33] = splits[s][:, :, :, c].transpose(
                0, 3, 4, 1, 2)

    # x81: [core, 81, hc, grp, 2, 34, GRP] -> [core, 81, NHC*NGRP*CHUNK]
    x81 = np.zeros((NCORES, 81, NHC, NGRP, 2, 34, GRP),
                   dtype=ml_dtypes.bfloat16)
    for du in range(3):
        for dv in range(3):
            for c in range(3):
                for s in range(3):
                    r = 9 * (3 * du + dv) + 3 * c + s
                    for hc in range(NHC):
                        sl = xpad[s, c][:, 2 * hc + du:2 * hc + du + 2,
                                        dv:dv + 34]
                        x81[:, r, hc] = sl.transpose(0, 3, 1, 2, 4)
    x81 = x81.reshape(NCORES, 81, -1)

    # conv1 fused weights [81, 128]
    s1dw = sgn(np.asarray(w1_dw, np.float32))[:, 0]        # [3,3,3]
    s1pw = sgn(np.asarray(w1_pw, np.float32))[:, :, 0, 0]  # [128,3]
    w1t = np.zeros((81, 128), dtype=ml_dtypes.bfloat16)
    for du in range(3):
        for dv in range(3):
            for c in range(3):
                for s in range(3):
                    w1t[9 * (3 * du + dv) + 3 * c + s] = _bf16(
                        s1pw[:, c] * s1dw[c, du, dv])
    b1eff = (sgn(np.asarray(b1_pw, np.float32))
             + s1pw @ sgn(np.asarray(b1_dw, np.float32))).astype(np.float32)
    negb1 = (-b1eff).astype(np.float32)
    sigb1 = (1e30 * b1eff).astype(np.float32)

    # dw: 9-tap diagonal bf16 (DoubleRow trips the HAM activity throttle)
    s2dw = sgn(np.asarray(w2_dw, np.float32))[:, 0]        # [128,3,3]
    dwt = np.zeros((128, 9 * 128), dtype=ml_dtypes.bfloat16)
    for t in range(9):
        np.fill_diagonal(dwt[:, t * 128:(t + 1) * 128],
                         _bf16(s2dw[:, t // 3, t % 3]))
    sdwb = sgn(np.asarray(b2_dw, np.float32))              # [128]

    # pw weights + fused threshold bias:
    # h1 stored as u in {0,1} (border 0.5, i.e. h1_true = 2u-1 with 0 border),
    # so true pw arg = 2*(pw @ dw_u) + C, C = sign(b2_pw) + pw @ (sdwb - S)
    # with S_c = sum of dw taps; h2 = sign(psum + C/2).
    s2pw = sgn(np.asarray(w2_pw, np.float32))[:, :, 0, 0]  # [256,128]
    pwt = _fp8(s2pw.T)                                     # [128,256]
    S_c = s2dw.sum(axis=(1, 2))                            # [128]
    b2h = (sgn(np.asarray(b2_pw, np.float32))
           + s2pw @ (sdwb - S_c)).astype(np.float32) / 2.0  # [256]
    b2h = b2h.reshape(2, 128).T.copy().astype(np.float32)  # [128, 2] mt cols
    sigb2 = (1e30 * b2h).astype(np.float32)

    # fc1 weights: wperm[feat, kt, c']
    fc1_w = np.asarray(fc1_w, np.float32)                  # [1024, 16384]
    cols = np.empty(KFC, np.int64)
    i = 0
    for ct in range(2):
        for x0_ in range(64):
            for cp in range(128):
                cols[i] = (ct * 128 + cp) * 64 + x0_
                i += 1
    wperm = fc1_w[:, cols].reshape(NF1, NKT, 128)
    whi = wperm.astype(np.float16)
    wlo = (wperm - whi.astype(np.float32)).astype(np.float16)
    # s1 threshold: true fc1 arg = 2*(W @ u2) - rowsum(W)
    wsumh = (wperm.astype(np.float64).sum(axis=(1, 2)) / 2.0).astype(
        np.float32)                                        # [1024]
    # wfc[core, blk, c', ktb, hl, m]
    wfc = np.empty((NCORES, NBLK, 128, 8, 2, 128), np.float16)
    for n in range(NCORES):
        f0 = n * FPC
        wh = whi[f0:f0 + 128]                              # [128m, kt, c']
        wl = wlo[f0:f0 + 128]
        wfc[n, :, :, :, 0] = wh.reshape(128, NBLK, 8, 128).transpose(
            1, 3, 2, 0)
        wfc[n, :, :, :, 1] = wl.reshape(128, NBLK, 8, 128).transpose(
            1, 3, 2, 0)
    wfc = wfc.reshape(NCORES, NBLK, 128, -1)

    # fc2 [core, featpart 128, hl, 10]
    fc2_w = np.asarray(fc2_w, np.float32)                  # [10, 1024]
    f2 = np.empty((NCORES, 128, 2, 10), np.float16)
    for n in range(NCORES):
        w = fc2_w[:, n * FPC:(n + 1) * FPC].T
        f2h = w.astype(np.float16)
        f2[n, :, 0] = f2h
        f2[n, :, 1] = (w - f2h.astype(np.float32)).astype(np.float16)
    f2 = f2.reshape(NCORES, 128, -1)

    shared = {
        "w1t": w1t, "negb1": negb1.reshape(128, 1),
        "sigb1": sigb1.reshape(128, 1),
        "dwt": dwt, "pwt": pwt, "sigb2": sigb2,
    }
    per_core = []
    for n in range(NCORES):
        d = dict(shared)
        d["x81"] = np.ascontiguousarray(x81[n])
        d["wfc"] = np.ascontiguousarray(wfc[n])
        d["f2"] = np.ascontiguousarray(f2[n])
        d["wsumh"] = np.ascontiguousarray(
            wsumh[n * FPC:(n + 1) * FPC].reshape(128, 1))
        per_core.append(d)
    return per_core


def _conv_half(nc, tc, ctx, grps, x81, w1_t, b1_t, sb1_t, dwp_t, pw_t, b2_t,
               h2t):
    """Conv pipeline for a list of groups, writing into h2t [128, 2*64*64].

    h1 is stored as u = (sign+1)/2 in {0,1} bf16->fp8, border 0.5; h2 as +-1.
    """
    impool = ctx.enter_context(tc.tile_pool(name="imt", bufs=8))
    h1pool = ctx.enter_context(tc.tile_pool(name="h1", bufs=2))
    dwcpool = ctx.enter_context(tc.tile_pool(name="dwc", bufs=2))
    mxpool = ctx.enter_context(tc.tile_pool(name="mx", bufs=6))
    qp = ctx.enter_context(tc.tile_pool(name="qp", bufs=4, space="PSUM"))

    for gi, g in enumerate(grps):
        g_loc = gi  # position within the half (0 or 1)
        # ---- conv1 + pool1 ----
        h1 = h1pool.tile([128, H1SZ], FP8, tag="h1")
        h1v = h1[:].rearrange("p (y x b) -> p y x b", y=H1Y, x=H1X)
        nc.vector.memset(h1v[:, 0], 0.5)
        nc.vector.memset(h1v[:, 17:19], 0.5)
        nc.vector.memset(h1v[:, 1:17, 0], 0.5)
        nc.vector.memset(h1v[:, 1:17, 17], 0.5)

        for hc in range(NHC):
            imt = impool.tile([81, CHUNK], BF16, tag="im")
            off = (hc * NGRP + g) * CHUNK
            nc.sync.dma_start(imt[:], x81[:, off:off + CHUNK])
            imv = imt[:].rearrange("p (h w2 two b) -> p h w2 two b",
                                   h=2, two=2, b=GRP)
            ts = []
            for dy in range(2):
                ps = qp.tile([128, 1024], F32, tag="qp")
                for k in range(2):
                    nc.tensor.matmul(ps[:, k * 512:(k + 1) * 512], w1_t[:],
                                     imv[:, dy, 0:16, k, :],
                                     start=True, stop=True)
                # u = (sign(psum + b1)+1)/2 in {0,1}; ACT via saturated
                # sigmoid, DVE via is_ge (ties are measure-zero here)
                u = mxpool.tile([128, 1024], BF16, tag="u")
                if hc % 5 < 3:
                    nc.scalar.activation(u[:], ps[:], AF.Sigmoid,
                                         bias=sb1_t[:], scale=1e30)
                else:
                    nc.vector.tensor_scalar(u[:], ps[:], b1_t[:], None,
                                            ALU.is_ge)
                t = mxpool.tile([128, 512], BF16, tag="t")
                nc.vector.tensor_max(t[:], u[:, 0:512], u[:, 512:1024])
                ts.append(t)
            nc.vector.tensor_max(h1v[:, hc + 1, 1:17, :],
                                 ts[0][:].rearrange("p (w b) -> p w b",
                                                    b=GRP),
                                 ts[1][:].rearrange("p (w b) -> p w b",
                                                    b=GRP))

        # ---- conv2 dw: 9-tap diagonal bf16, tap-major weight reuse ----
        dwc = dwcpool.tile([128, DWSZ], BF16, tag="dwc")
        for ybb in range(4):
            pss = [qp.tile([128, 1024], F32, tag="qp", name=f"dw{ybb}{k}")
                   for k in range(2)]
            for t in range(9):
                du, dv = t // 3, t % 3
                for yy in range(4):
                    y = ybb * 4 + yy
                    mv = h1v[:, y + du, dv:dv + 16, :]
                    nc.tensor.matmul(
                        pss[yy // 2][:, (yy % 2) * 512:(yy % 2 + 1) * 512],
                        dwp_t[:, t * 128:(t + 1) * 128], mv,
                        start=(t == 0), stop=(t == 8))
            for k in range(2):
                nc.scalar.copy(
                    dwc[:, (ybb * 2 + k) * 1024:(ybb * 2 + k + 1) * 1024],
                    pss[k][:])

        # ---- conv2 pw + pool2 (h2 as +-1, threshold bias folded) ----
        dwv = dwc[:].rearrange("p (y2 dy x2 dx b) -> p y2 dy x2 dx b",
                               y2=8, dy=2, dx=2, b=GRP)
        h2view = h2t[:].rearrange("p (mt pos b) -> p mt pos b",
                                  mt=2, b=2 * GRP)
        for mt in range(2):
            for y2b in range(4):
                ts2 = []
                for dy in range(2):
                    ps = qp.tile([128, 1024], F32, tag="qp")
                    for dx in range(2):
                        nc.tensor.matmul(
                            ps[:, dx * 512:(dx + 1) * 512],
                            pw_t[:, mt * 128:(mt + 1) * 128],
                            dwv[:, 2 * y2b:2 * y2b + 2, dy, :, dx, :],
                            start=True, stop=True)
                    sq = mxpool.tile([128, 1024], BF16, tag="u")
                    nc.scalar.activation(sq[:], ps[:], AF.Sigmoid,
                                         bias=b2_t[:, mt:mt + 1], scale=1e30)
                    t = mxpool.tile([128, 512], BF16, tag="t")
                    nc.vector.tensor_max(t[:], sq[:, 0:512], sq[:, 512:1024])
                    ts2.append(t)
                nc.vector.tensor_max(
                    h2view[:, mt, y2b * 16:(y2b + 1) * 16,
                           g_loc * GRP:(g_loc + 1) * GRP],
                    ts2[0][:].rearrange("p (q b) -> p q b", b=GRP),
                    ts2[1][:].rearrange("p (q b) -> p q b", b=GRP))


def build_program():
    nc = bacc.Bacc("TRN2", target_bir_lowering=False, debug=False,
                   num_devices=NCORES)

    def din(name, shape, dt):
        return nc.dram_tensor(name, shape, dt, kind="ExternalInput").ap()

    x81 = din("x81", [81, NHC * NGRP * CHUNK], BF16)
    w1t = din("w1t", [81, 128], BF16)
    negb1 = din("negb1", [128, 1], F32)
    sigb1 = din("sigb1", [128, 1], F32)
    dwt = din("dwt", [128, 9 * 128], BF16)
    pwt = din("pwt", [128, 256], FP8)
    sigb2 = din("sigb2", [128, 2], F32)
    wsumh = din("wsumh", [128, 1], F32)
    wfc = din("wfc", [NBLK, 128, 2048], FP16)
    f2 = din("f2", [128, 20], FP16)
    y_out = nc.dram_tensor("y", [10, NBG], F32, kind="ExternalOutput").ap()

    hsh = [nc.dram_tensor(f"h2_shard_{h}", [2, 128, 4096], FP8).ap()
           for h in range(2)]
    hall = [nc.dram_tensor(f"h2_all_{h}", [NCORES, 2, 128, 4096], FP8,
                           addr_space="Shared").ap() for h in range(2)]
    groups = [list(range(NCORES))]

    with ExitStack() as octx:
        cc_sems = [octx.enter_context(nc.semaphore(f"cc_{h}"))
                   for h in range(2)]

        for half in range(2):
            with tile.TileContext(nc) as tc, ExitStack() as ctx:
                cpool = ctx.enter_context(tc.tile_pool(name="c", bufs=1))
                w1_t = cpool.tile([81, 128], BF16)
                nc.sync.dma_start(w1_t[:], w1t[:])
                b1_t = cpool.tile([128, 1], F32)
                nc.sync.dma_start(b1_t[:], negb1[:])
                sb1_t = cpool.tile([128, 1], F32)
                nc.sync.dma_start(sb1_t[:], sigb1[:])
                dwp_t = cpool.tile([128, 9 * 128], BF16)
                nc.sync.dma_start(dwp_t[:], dwt[:])
                pw_t = cpool.tile([128, 256], FP8)
                nc.sync.dma_start(pw_t[:], pwt[:])
                b2_t = cpool.tile([128, 2], F32)
                nc.sync.dma_start(b2_t[:], sigb2[:])
                h2t = cpool.tile([128, 2 * 64 * 2 * GRP], FP8)

                # HAM warmup while first im2col DMAs land
                with tc.tile_pool(name="wm", bufs=1) as wmp, \
                     tc.tile_pool(name="wmp", bufs=1, space="PSUM") as wps:
                    wmt = wmp.tile([128, 512], FP8)
                    nc.vector.memset(wmt[:], 1.0)
                    wp_t = wps.tile([128, 512], F32)
                    n_wm = 24 if half == 0 else 8
                    for w in range(n_wm):
                        nc.tensor.matmul(wp_t[:], wmt[:, 0:128], wmt[:],
                                         start=(w == 0), stop=(w == n_wm - 1))

                _conv_half(nc, tc, ctx, [2 * half, 2 * half + 1],
                           x81, w1_t, b1_t, sb1_t, dwp_t, pw_t, b2_t, h2t)
                for mt in range(2):
                    nc.sync.dma_start(hsh[half][mt],
                                      h2t[:, mt * 4096:(mt + 1) * 4096])

            # fire AllGather for this half (wait deferred)
            _sh, _al, _sem = hsh[half], hall[half], cc_sems[half]
            with nc.Block() as blk:
                @blk.gpsimd
                def _(gp):
                    gp.collective_compute(
                        "AllGather", ALU.bypass, replica_groups=groups,
                        ins=[_sh], outs=[_al],
                    ).then_inc(_sem)

        with nc.Block() as blk:
            @blk.gpsimd
            def _(gp):
                gp.wait_ge(cc_sems[0], 1)
                gp.wait_ge(cc_sems[1], 1)
        nc.all_engine_barrier()

        # ---- fc phase ----
        with tile.TileContext(nc) as tc, ExitStack() as ctx:
            sp = ctx.enter_context(tc.tile_pool(name="fcs", bufs=1))
            wp = ctx.enter_context(tc.tile_pool(name="wfc", bufs=3))
            psp = ctx.enter_context(tc.tile_pool(name="psf", bufs=1,
                                                 space="PSUM"))
            wmps = ctx.enter_context(tc.tile_pool(name="fwm", bufs=1,
                                                  space="PSUM"))

            # re-warm PE after gather idle (no DMA dependency)
            wmt = sp.tile([128, 512], FP8)
            nc.vector.memset(wmt[:], 1.0)
            wp_t = wmps.tile([128, 512], F32)
            for w in range(48):
                nc.tensor.matmul(wp_t[:], wmt[:, 0:128], wmt[:],
                                 start=(w == 0), stop=(w == 47))

            # weight blocks 0-1 first so fc1 can start as soon as ct0 lands
            wt01 = []
            for b0 in range(2):
                wt = wp.tile([128, 2048], FP16, tag="w", name=f"wt{b0}")
                nc.sync.dma_start(wt[:], wfc[b0])
                wt01.append(wt)
            hg = {}
            for ct in range(2):
                for hf in range(2):
                    t = sp.tile([128, GSIZE * 4096], FP8,
                                tag=f"hg{ct}{hf}", name=f"hg{ct}{hf}")
                    for s in range(GSIZE):
                        nc.sync.dma_start(t[:, s * 4096:(s + 1) * 4096],
                                          hall[hf][s, ct])
                    hg[(ct, hf)] = t

            f2_t = sp.tile([128, 20], FP16)
            nc.sync.dma_start(f2_t[:], f2[:])
            psf = [psp.tile([128, 512], F32, tag=f"psf{hf}", name=f"psf{hf}")
                   for hf in range(2)]

            for blk_i in range(NBLK):
                if blk_i < 2:
                    wt = wt01[blk_i]
                else:
                    wt = wp.tile([128, 2048], FP16, tag="w")
                    nc.sync.dma_start(wt[:], wfc[blk_i])
                for ktb in range(8):
                    kt = blk_i * 8 + ktb
                    ct, xx = kt // 64, kt % 64
                    for hl in range(2):
                        lhs = wt[:, (ktb * 2 + hl) * 128:
                                 (ktb * 2 + hl + 1) * 128]
                        for hf in range(2):
                            rhs = hg[(ct, hf)][:].rearrange(
                                "p (s x b) -> p s x b", s=GSIZE,
                                b=2 * GRP)[:, :, xx, :]
                            nc.tensor.matmul(
                                psf[hf][:], lhs, rhs,
                                start=(kt == 0 and hl == 0),
                                stop=(kt == NKT - 1 and hl == 1))

            ws_t = sp.tile([128, 1], F32)
            nc.sync.dma_start(ws_t[:], wsumh[:])
            s1 = sp.tile([128, 1024], FP8)
            for hf in range(2):
                nc.vector.tensor_scalar(s1[:, hf * 512:(hf + 1) * 512],
                                        psf[hf][:], ws_t[:], None, ALU.is_ge)
            ps10 = psp.tile([10, 1024], F32, tag="ps10")
            f2v = f2_t[:].rearrange("p (hl o) -> p hl o", hl=2)
            k = 0
            for hl in range(2):
                for hf in range(2):
                    nc.tensor.matmul(ps10[:, hf * 512:(hf + 1) * 512],
                                     f2v[:, hl],
                                     s1[:, hf * 512:(hf + 1) * 512],
                                     start=(hl == 0), stop=(hl == 1))
                    k += 1
            yt = sp.tile([10, NBG], F32)
            nc.scalar.copy(yt[:], ps10[:])
            nc.sync.dma_start(y_out[:], yt[:])

    nc.compile()
    return nc


_CACHE = {}


def _get_program():
    if "nc" not in _CACHE:
        _CACHE["nc"] = build_program()
    return _CACHE["nc"]


def kernel(**inputs):
    per_core = _host_prep(**inputs)
    nc = _get_program()
    res = run_bass_kernel_spmd(nc, per_core, core_ids=list(range(NCORES)))
    fc2_b = np.asarray(inputs["fc2_b"], np.float32)
    # device column j = hf*512 + s*64 + b  ->  image s*128 + hf*64 + b
    perm = np.empty(NBG, np.int64)
    for hf in range(2):
        for s in range(GSIZE):
            for b in range(64):
                perm[hf * 512 + s * 64 + b] = s * 128 + hf * 64 + b
    y = np.zeros((NB_ALL, 10), np.float32)
    for n in range(NCORES):
        y[perm] += res.results[n]["y"].T
    f2sum = np.asarray(inputs["fc2_w"], np.float64).sum(axis=1).astype(
        np.float32)
    return (2.0 * y - f2sum[None, :] + fc2_b[None, :]).astype(np.float32)
